# revision 43
# baseline (speedup 1.0000x reference)
"""Trainium2 Bass kernel for nn_Attention_pps (dense_transformer).

Mathematical reduction of the reference:
  - x_pps has N=1, so attn = softmax over a length-1 axis == 1.0 exactly.
  - Therefore out = v_img, and the whole module collapses to one affine map:
        out = x[:, 0, :] @ (W_kv[:, C:] @ W_proj) + b_proj
  - W_c = W_kv[:, C:] @ W_proj is fused on host in float64 (512x512, trivial).

Device strategy (8 NeuronCores, pure data parallel over batch):
  - Each core gets 8192 rows of x_img, shipped as bf16 (tolerance is 2e-2;
    bf16 end-to-end costs ~3e-3) pre-packed on host into the exact SBUF tile
    layout ([chunk][128 part][kt][m]) so every DMA is a long contiguous run
    per partition and the contraction dim lands on SBUF partitions with no
    on-chip transposes.
  - Per core: one GEMM [8192x512] @ [512x512]; bf16 matmuls (full-rate PE,
    FWL weight loads) accumulate in fp32 PSUM; eviction is a DVE tensor_copy
    straight to bf16. The bias add moves to the host epilogue (fp32-exact).
  - bf16 halves DMA traffic vs fp32 (8 MiB in + 8 MiB out per core), making
    the kernel PE-bound (~55 us of matmul rows at 2.4 GHz).
  - Startup: W_c k-tiles and the (small) first chunks interleave on the two
    low-latency HWDGE rings (sync, scalar); a continuous stream of small
    warm-up matmuls keeps the PE busy from the preamble until real data
    lands, so the HAM clock gate reaches 8/8 as real matmuls begin.
  - gpsimd (SWDGE) only carries mid-kernel stores, so its slow teardown
    drain overlaps compute instead of extending the epilogue.
  - Tail chunks shrink (384/256/128 rows) and evict on two engines with
    per-half stores so the drain after the last matmul stays ~1 us.
"""

import numpy as np
import ml_dtypes

BF16 = ml_dtypes.bfloat16

B = 65536
C = 512
N_CORES = 8
M_PER_CORE = B // N_CORES  # 8192
KT = C // 128              # 4 k-tiles

# chunk sizes (rows); small at both ends to shorten pipeline ramp/drain
CHUNKS = [128, 128, 256, 256] + [512] * 13 + [384, 256, 128]
assert sum(CHUNKS) == M_PER_CORE

_COMPILED = None


def _build():
    from concourse import bacc, tile, mybir

    nc = bacc.Bacc("TRN2", target_bir_lowering=False, debug=False)
    f32 = mybir.dt.float32
    bf16 = mybir.dt.bfloat16

    total = M_PER_CORE * C
    xp = nc.dram_tensor("xp", [total], bf16, kind="ExternalInput")
    wc = nc.dram_tensor("wc", [C, C], bf16, kind="ExternalInput")
    op = nc.dram_tensor("op", [total], bf16, kind="ExternalOutput")

    with tile.TileContext(nc) as tc:
        with (
            tc.tile_pool(name="consts", bufs=1) as consts,
            tc.tile_pool(name="xin", bufs=12) as xin,
            tc.tile_pool(name="outp", bufs=12) as outp,
            tc.tile_pool(name="psum", bufs=2, space="PSUM") as psum,
        ):
            hw = [nc.sync, nc.scalar]  # low-latency HWDGE rings

            # Wc k-tiles land as the first DMA on each HWDGE ring (kt0 on
            # sync, kt2 on scalar) so chunk 0's first matmuls aren't gated on
            # the whole 512 KiB weight load; kt1/kt3 follow chunk 0's x.
            wc_sb = consts.tile([128, KT, C], bf16)

            def load_wc(kt, ring):
                ring.dma_start(
                    out=wc_sb[:, kt, :], in_=wc[kt * 128 : (kt + 1) * 128, :]
                )

            load_wc(0, hw[0])
            load_wc(2, hw[1])

            # PE warm-up: continuous stream of small dummy matmuls with no DMA
            # deps (memset on DVE). They bridge the preamble + first-load
            # window so the HAM activity monitor sees sustained PE busy and
            # lifts the clock gate to 8/8 by the time real matmuls start.
            warm_w = consts.tile([128, 128], bf16)
            warm_x = consts.tile([128, 256], bf16)
            nc.vector.memset(warm_w[:], 0.0)
            nc.vector.memset(warm_x[:], 0.0)
            # full-bank psum tile so later accumulators stay bank-aligned
            warm_ps = psum.tile([128, C], f32, tag="acc")
            N_WARM = 16
            for i in range(N_WARM):
                nc.tensor.matmul(
                    warm_ps[:, :256],
                    warm_w[:],
                    warm_x[:],
                    start=(i == 0),
                    stop=(i == N_WARM - 1),
                )

            m0 = 0
            n_chunks = len(CHUNKS)
            for ci, L in enumerate(CHUNKS):
                nt = L // 128  # m-tiles in this chunk
                boff = m0 * C  # flat element offset of this chunk's block

                # load x^T chunk: [128 (k within tile), kt, m], split across
                # the two HWDGE rings (kt 0-1 / kt 2-3) for parallel draw
                xt_sb = xin.tile([128, KT, L], bf16, tag="xin")
                half = 128 * 2 * L
                for h in range(2):
                    hw[(ci + h) % 2].dma_start(
                        out=xt_sb[:, 2 * h : 2 * h + 2, :],
                        in_=xp[boff + h * half : boff + (h + 1) * half].rearrange(
                            "(p kt m) -> p kt m", p=128, kt=2
                        ),
                    )
                if ci == 0:
                    # remaining wc k-tiles, right behind chunk 0's halves
                    load_wc(1, hw[0])
                    load_wc(3, hw[1])

                out_sb = outp.tile([128, nt, C], bf16, tag="outp")
                acc = psum.tile([128, nt, C], f32, tag="acc")
                # kt-inner: back-to-back accumulation into the same PSUM
                # bank is the PE fast path (measured 379 vs 454 ns/MM)
                for ms in range(nt):
                    for kt in range(KT):
                        nc.tensor.matmul(
                            acc[:, ms, :],
                            xt_sb[:, kt, ms * 128 : (ms + 1) * 128],
                            wc_sb[:, kt, :],
                            start=(kt == 0),
                            stop=(kt == KT - 1),
                        )
                op_ap = op[boff : boff + 128 * nt * C].rearrange(
                    "(p s n) -> p s n", p=128, s=nt
                )
                if ci >= n_chunks - 3:
                    # tail: evict on two engines and store each half on its
                    # own HWDGE ring so the final drain is parallel
                    half_n = C // 2
                    nc.vector.tensor_copy(out_sb[:, :, :half_n], acc[:, :, :half_n])
                    nc.scalar.copy(out_sb[:, :, half_n:], acc[:, :, half_n:])
                    hw[ci % 2].dma_start(
                        out=op_ap[:, :, :half_n], in_=out_sb[:, :, :half_n]
                    )
                    hw[(ci + 1) % 2].dma_start(
                        out=op_ap[:, :, half_n:], in_=out_sb[:, :, half_n:]
                    )
                else:
                    nc.vector.tensor_copy(out_sb[:], acc[:])
                    if 2 <= ci < n_chunks - 3:
                        # mid-kernel stores ride the gpsimd SWDGE ring; its
                        # slow teardown drain then overlaps compute
                        nc.gpsimd.dma_start(out=op_ap[:], in_=out_sb[:])
                    else:
                        hw[ci % 2].dma_start(out=op_ap[:], in_=out_sb[:])
                m0 += L

    nc.compile()
    return nc


def _get_compiled():
    global _COMPILED
    if _COMPILED is None:
        _COMPILED = _build()
    return _COMPILED


def _pack_shard(shard):
    """shard: [M_PER_CORE, C] fp32 (x_img rows for one core) -> flat bf16 blob.
    Per chunk: two half-blocks [128 p][2 kt][m] (kt 0-1 then kt 2-3), matching
    the two split load DMAs."""
    shard = shard.astype(BF16)
    blocks = []
    m0 = 0
    for L in CHUNKS:
        blk = shard[m0 : m0 + L, :].T.reshape(KT, 128, L)  # [kt, p, m]
        for h in range(2):
            half = blk[2 * h : 2 * h + 2].transpose(1, 0, 2)  # [p, 2, m]
            blocks.append(np.ascontiguousarray(half).reshape(-1))
        m0 += L
    return np.concatenate(blocks)


def _unpack_out(flat):
    """Inverse of the store layout: flat [M_PER_CORE*C] -> [M_PER_CORE, C]."""
    rows = []
    m0 = 0
    for L in CHUNKS:
        nt = L // 128
        blk = flat[m0 * C : (m0 + L) * C].reshape(128, nt, C)
        rows.append(blk.transpose(1, 0, 2).reshape(L, C))
        m0 += L
    return np.concatenate(rows, axis=0)


def _prep_in_maps(x, W_kv, W_proj):
    x = np.asarray(x, dtype=np.float32)
    W_kv = np.asarray(W_kv, dtype=np.float32)
    W_proj = np.asarray(W_proj, dtype=np.float32)

    wc_f = W_kv[:, C:].astype(np.float64) @ W_proj.astype(np.float64)
    wc = np.ascontiguousarray(wc_f.astype(BF16))

    x_img = x[:, 0, :]  # [B, C] (strided view)
    in_maps = []
    for c in range(N_CORES):
        shard = x_img[c * M_PER_CORE : (c + 1) * M_PER_CORE]
        in_maps.append({"xp": _pack_shard(shard), "wc": wc})
    return in_maps


def _run(inputs, trace=False):
    from concourse.bass_utils import run_bass_kernel_spmd

    nc = _get_compiled()
    in_maps = _prep_in_maps(inputs["x"], inputs["W_kv"], inputs["W_proj"])
    res = run_bass_kernel_spmd(nc, in_maps, core_ids=list(range(N_CORES)), trace=trace)
    b_proj = np.asarray(inputs["b_proj"], dtype=np.float32)
    parts = [
        _unpack_out(res.results[c]["op"]).astype(np.float32) for c in range(N_CORES)
    ]
    full = (np.concatenate(parts, axis=0) + b_proj).reshape(B, 1, C)
    return full, res


def kernel(x, W_kv, W_proj, b_proj):
    out, _ = _run({"x": x, "W_kv": W_kv, "W_proj": W_proj, "b_proj": b_proj})
    return out
